# revision 38
# baseline (speedup 1.0000x reference)
"""ChunkwiseRetentionMixer Trainium2 kernel.

Computes: out = rms_norm(cumsum(x @ Ws^T, axis=L)) @ Wo^T  for x (B,L,H)=(4,8192,1024),
Ws (64,1024), Wo (1024,64), all float32.

Sharding: 8 cores = (batch, L-half) pairs. Each core processes a [4096, 1024]
chunk. The cumsum carry into second-half chunks is seeded from a column-sum of
the first half of x (computed on host during sharding; 0.4% of the FLOPs).

Per-core dataflow:
  x block [128l, 128h] --PE transpose--> xT [128h, 128l] (PSUM) --evac--> SBUF
  state_nat [128l, 64s] = sum_h xT_h(stationary) @ WsT_h(moving)   (PE, fp32;
      flipped so the fp32 4-cyc/row cost applies to only 64 output columns)
  state_nat --PE transpose--> stateT [64s, 128l] --evac--> SBUF
  cumT = tensor_tensor_scan(stateT, add)       (DVE; carry chains via initial)
  ms[128l, 1] = (cumT_block^2)-colsum via PE   (lhsT = sq block, rhs = ones)
  rstd = 1/sqrt(ms/64 + eps)                   (ACT sqrt + DVE reciprocal)
  out block [128l, 1024h] = cumT_block.T @ WoT (PE; fp32, single f32r, or
      3-pass f32r-residual = full-rate with ~1e-8 error), evacuated PSUM->SBUF
      with per-row rstd as the activation-copy scale (rms_norm commutes).
"""

import os
import numpy as np

B, L, H, S = 4, 8192, 1024, 64
LC = L // 2            # rows per core chunk
P = 128                # partitions / block rows
NBLK = LC // P         # 32 l-blocks per core
GROUP_BLOCKS = 4       # l-blocks per pipeline group
GROUP_ROWS = GROUP_BLOCKS * P      # 512
NGROUPS = NBLK // GROUP_BLOCKS     # 8
KH = H // P            # 8 h-tiles of 128
EPS = 1.1920928955078125e-07

DEFAULT_CFG = dict(
    out_mode="f32rk",   # "f32" | "f32r" | "f32r3" | "f32rk"
    bufs_xin=3, bufs_xtp=3, bufs_outp=4, bufs_small=4,
    ps_xt=3, ps_stn=1, ps_stt=1, ps_ms=1, ps_out=2,
    group_blocks=4,     # l-blocks per pipeline group
    out_split=2,        # out DMA stores per group (1, 2, or 4)
    xt_evac=1,          # 0=all DVE, 1=all ACT, 2=alternate
)

_CACHE = {}


def _emit(nc, tc, ctx, aps, cfg, reps=1):
    import concourse.bass as bass
    from concourse import mybir

    f32 = mybir.dt.float32
    f32r = mybir.dt.float32r
    out_mode = cfg["out_mode"]
    dt_out = f32 if out_mode == "f32" else f32r
    GB = cfg.get("group_blocks", GROUP_BLOCKS)
    GR = GB * P
    NG = NBLK // GB
    x_d, pf_d, ws_d, wo_d, id_d, out_d = (
        aps["x"], aps["pf"], aps["ws"], aps["wo"], aps["ident"], aps["out"])

    singles = ctx.enter_context(tc.tile_pool(name="singles", bufs=1))
    small = ctx.enter_context(tc.tile_pool(name="small", bufs=cfg["bufs_small"]))

    # ---- constants / weight prep -------------------------------------------
    ident = singles.tile([P, P], f32)
    nc.sync.dma_start(out=ident[:], in_=id_d[:, :])

    ws_sb = singles.tile([S, H], f32)
    nc.sync.dma_start(out=ws_sb[:], in_=ws_d[:, :])
    wo_sb = singles.tile([P, KH, S], f32)   # Wo rows tiled: [128, 8, 64]
    nc.sync.dma_start(
        out=wo_sb[:], in_=wo_d[:, :].rearrange("(k p) s -> p k s", p=P))
    pfT = singles.tile([P, KH], f32)        # prefix colsum, h on partitions
    nc.sync.dma_start(out=pfT[:], in_=pf_d[:, :])

    wsT = singles.tile([P, KH * S], f32)    # WsT: h-tile i at [64i, 64i+64)
    woT = singles.tile([S, H], dt_out, name="woT")   # WoT [64s, 1024h]
    woT_f = woT if out_mode == "f32" else singles.tile([S, H], f32, name="woT_f")
    dwoT = (singles.tile([S, H], f32r, name="dwoT")
            if out_mode == "f32r3" else None)
    if out_mode == "f32rk":
        # [Wo_r; Wo_r] and [dWo_r; 0] stacked along K for the merged
        # residual out-projection
        wow = singles.tile([P, H], f32r, name="wow")
        dwoz = singles.tile([P, H], f32r, name="dwoz")
    ones_col = singles.tile([S, 1], f32)
    nc.vector.memset(ones_col[:], 1.0)
    eps_col = singles.tile([P, 1], f32)
    nc.vector.memset(eps_col[:], EPS)

    c0 = small.tile([S, 1], f32)
    with tc.tile_pool(name="ps_prep", bufs=2, space="PSUM") as ps_prep:
        for i in range(KH):
            t_ps = ps_prep.tile([P, P], f32, tag="prep")
            nc.tensor.transpose(
                t_ps[:, 0:S], ws_sb[:, i * P:(i + 1) * P], ident[0:S, 0:S])
            nc.vector.tensor_copy(wsT[:, i * S:(i + 1) * S], t_ps[:, 0:S])
        for i in range(KH):
            t_ps = ps_prep.tile([P, P], f32, tag="prep")
            nc.tensor.transpose(t_ps[0:S, :], wo_sb[:, i, :], ident[:, :])
            nc.vector.tensor_copy(woT_f[:, i * P:(i + 1) * P], t_ps[0:S, :])
        if out_mode != "f32":
            nc.scalar.activation(
                woT[:], woT_f[:], mybir.ActivationFunctionType.Copy)
        if out_mode == "f32r3":
            nc.vector.tensor_sub(dwoT[:], woT_f[:], woT[:].bitcast(f32))
        if out_mode == "f32rk":
            nc.scalar.activation(
                wow[0:S, :], woT_f[:], mybir.ActivationFunctionType.Copy)
            nc.scalar.activation(
                wow[S:P, :], woT_f[:], mybir.ActivationFunctionType.Copy)
            nc.vector.tensor_sub(
                dwoz[0:S, :], woT_f[:], wow[0:S, :].bitcast(f32))
            nc.vector.tensor_sub(dwoz[S:P, :], woT_f[:], woT_f[:])

        # carry0 = pf @ Ws^T, shape [64, 1] (zero for first-half cores)
        c0_ps = ps_prep.tile([P, P], f32, tag="prep")
        for i in range(KH):
            nc.tensor.matmul(
                c0_ps[0:S, 0:1], wsT[:, i * S:(i + 1) * S], pfT[:, i:i + 1],
                start=(i == 0), stop=(i == KH - 1))
        nc.vector.tensor_copy(c0[:], c0_ps[0:S, 0:1])

    xin = ctx.enter_context(tc.tile_pool(name="xin", bufs=cfg["bufs_xin"]))
    xtp = ctx.enter_context(tc.tile_pool(name="xtp", bufs=cfg["bufs_xtp"]))
    outp = ctx.enter_context(tc.tile_pool(name="outp", bufs=cfg["bufs_outp"]))
    ps_xt = ctx.enter_context(
        tc.tile_pool(name="ps_xt", bufs=cfg["ps_xt"], space="PSUM"))
    ps_stn = ctx.enter_context(
        tc.tile_pool(name="ps_stn", bufs=cfg["ps_stn"], space="PSUM"))
    ps_stt = ctx.enter_context(
        tc.tile_pool(name="ps_stt", bufs=cfg["ps_stt"], space="PSUM"))
    ps_ms = ctx.enter_context(
        tc.tile_pool(name="ps_ms", bufs=cfg["ps_ms"], space="PSUM"))
    ps_out = ctx.enter_context(
        tc.tile_pool(name="ps_out", bufs=cfg["ps_out"], space="PSUM"))

    carry = c0
    carry_col = 0
    for g in range(NG * reps):
        g = g % NG
        rows = slice(g * GR, (g + 1) * GR)
        x_sb = xin.tile([P, GB, H], f32)
        nc.sync.dma_start(
            out=x_sb[:],
            in_=x_d[rows, :].rearrange("(n p) h -> p n h", p=P))

        # transpose x 128x128 tiles; xT_sb free = (n, i, l)
        xt_sb = xtp.tile([P, GB * H], f32)
        for n in range(GB):
            for half in range(2):
                xt_ps = ps_xt.tile([P, 4 * P], f32)
                for j in range(4):
                    i = half * 4 + j
                    nc.tensor.transpose(
                        xt_ps[:, j * P:(j + 1) * P],
                        x_sb[:, n, i * P:(i + 1) * P],
                        ident[:, :])
                use_dve = (cfg["xt_evac"] == 0 or
                           (cfg["xt_evac"] == 2 and (n + half) % 2 == 0))
                dst = xt_sb[:, n * H + half * 4 * P: n * H + (half + 1) * 4 * P]
                if use_dve:
                    nc.vector.tensor_copy(dst, xt_ps[:])
                else:
                    nc.scalar.activation(
                        dst, xt_ps[:], mybir.ActivationFunctionType.Copy)

        # flipped projection per block: state_nat [128l, 64s], then PE
        # transpose to stateT and assemble the group's [64, 512]
        st_sb = small.tile([S, GR], f32)
        for n in range(GB):
            stn_ps = ps_stn.tile([P, S], f32)
            for i in range(KH):
                nc.tensor.matmul(
                    stn_ps[:],
                    xt_sb[:, n * H + i * P:n * H + (i + 1) * P],
                    wsT[:, i * S:(i + 1) * S],
                    start=(i == 0), stop=(i == KH - 1))
            stn_sb = small.tile([P, S], f32)
            nc.vector.tensor_copy(stn_sb[:], stn_ps[:])
            stt_ps = ps_stt.tile([S, P], f32)
            nc.tensor.transpose(stt_ps[:], stn_sb[:], ident[:, :])
            nc.vector.tensor_copy(st_sb[:, n * P:(n + 1) * P], stt_ps[:])

        # running cumsum along l with carry chaining
        cum_sb = small.tile([S, GR], f32)
        nc.vector.tensor_tensor_scan(
            cum_sb[:], st_sb[:], st_sb[:], carry[:, carry_col:carry_col + 1],
            mybir.AluOpType.add, mybir.AluOpType.bypass)
        carry, carry_col = cum_sb, GR - 1

        if out_mode == "f32rk":
            # [cum_r; dcum_r] stacked along partitions as one K=128 operand
            cumcat = small.tile([P, GR], f32r)
            nc.scalar.activation(
                cumcat[0:S, :], cum_sb[:], mybir.ActivationFunctionType.Copy)
            nc.vector.tensor_sub(
                cumcat[S:P, :], cum_sb[:], cumcat[0:S, :].bitcast(f32))
        elif out_mode != "f32":
            cum_r = small.tile([S, GR], f32r)
            nc.scalar.activation(
                cum_r[:], cum_sb[:], mybir.ActivationFunctionType.Copy)
        if out_mode == "f32r3":
            dcum_r = small.tile([S, GR], f32r)
            nc.vector.tensor_sub(dcum_r[:], cum_sb[:], cum_r[:].bitcast(f32))

        osplit = cfg["out_split"]
        sub_blocks = GB // osplit
        out_tiles = []
        for _ in range(osplit):
            out_sb = outp.tile([P, sub_blocks, H], f32, tag="out_sb")
            out_tiles.append(out_sb)
        for n in range(GB):
            out_sb = out_tiles[n // sub_blocks]
            n_loc = n % sub_blocks
            blk = cum_sb[:, n * P:(n + 1) * P]          # [64, 128]
            sq = small.tile([S, P], f32)
            nc.vector.tensor_mul(sq[:], blk, blk)
            ms_ps = ps_ms.tile([P, 1], f32)
            nc.tensor.matmul(ms_ps[:], sq[:], ones_col[:], start=True, stop=True)
            rstd = small.tile([P, 1], f32)
            nc.scalar.activation(
                rstd[:], ms_ps[:], mybir.ActivationFunctionType.Sqrt,
                bias=eps_col[:], scale=1.0 / S)
            nc.vector.reciprocal(rstd[:], rstd[:])

            cols = slice(n * P, (n + 1) * P)
            for m in range(2):
                wcols = slice(m * (H // 2), (m + 1) * (H // 2))
                o_ps = ps_out.tile([P, H // 2], f32)
                if out_mode == "f32":
                    nc.tensor.matmul(
                        o_ps[:], cum_sb[:, cols], woT[:, wcols],
                        start=True, stop=True)
                elif out_mode == "f32r":
                    nc.tensor.matmul(
                        o_ps[:], cum_r[:, cols], woT[:, wcols],
                        start=True, stop=True)
                elif out_mode == "f32rk":
                    nc.tensor.matmul(
                        o_ps[:], cumcat[:, cols], wow[:, wcols],
                        start=True, stop=False)
                    nc.tensor.matmul(
                        o_ps[:], cumcat[:, cols], dwoz[:, wcols],
                        start=False, stop=True)
                else:
                    with tc.tile_critical():
                        nc.tensor.matmul(
                            o_ps[:], cum_r[:, cols], woT[:, wcols],
                            start=True, stop=False)
                        nc.tensor.matmul(
                            o_ps[:], dcum_r[:, cols], woT[:, wcols],
                            start=False, stop=False)
                        nc.tensor.matmul(
                            o_ps[:], cum_r[:, cols], dwoT[:, wcols],
                            start=False, stop=True)
                nc.scalar.activation(
                    out_sb[:, n_loc, wcols],
                    o_ps[:], mybir.ActivationFunctionType.Copy,
                    bias=0.0, scale=rstd[:])
            if n % sub_blocks == sub_blocks - 1:
                k = n // sub_blocks
                r0 = g * GR + k * sub_blocks * P
                nc.sync.dma_start(
                    out=out_d[r0:r0 + sub_blocks * P, :].rearrange(
                        "(n p) h -> p n h", p=P),
                    in_=out_tiles[k][:])


def _build(reps=1, cfg=None):
    cfg = {**DEFAULT_CFG, **(cfg or {})}
    key = ("nc", reps, tuple(sorted(cfg.items())))
    if key in _CACHE:
        return _CACHE[key]
    from contextlib import ExitStack
    import concourse.bacc as bacc
    import concourse.tile as tile
    from concourse import mybir

    f32 = mybir.dt.float32
    nc = bacc.Bacc("TRN2", target_bir_lowering=False, debug=False,
                   num_devices=8)
    aps = {
        "x": nc.dram_tensor("x", [LC, H], f32, kind="ExternalInput"),
        "pf": nc.dram_tensor("pf", [P, KH], f32, kind="ExternalInput"),
        "ws": nc.dram_tensor("ws", [S, H], f32, kind="ExternalInput"),
        "wo": nc.dram_tensor("wo", [H, S], f32, kind="ExternalInput"),
        "ident": nc.dram_tensor("ident", [P, P], f32, kind="ExternalInput"),
        "out": nc.dram_tensor("out", [LC, H], f32, kind="ExternalOutput"),
    }
    with tile.TileContext(nc) as tc:
        with ExitStack() as ctx:
            _emit(nc, tc, ctx, aps, cfg, reps=reps)
    nc.compile()
    _CACHE[key] = nc
    return nc


def kernel(x, Ws, Wo, _trace=False, _cfg=None, _trace_kwargs=None):
    from concourse.bass_utils import run_bass_kernel_spmd

    nc = _build(cfg=_cfg)
    x = np.ascontiguousarray(np.asarray(x, dtype=np.float32))
    ws = np.ascontiguousarray(np.asarray(Ws, dtype=np.float32))
    wo = np.ascontiguousarray(np.asarray(Wo, dtype=np.float32))
    ident = np.eye(P, dtype=np.float32)

    in_maps = []
    for c in range(8):
        b, half = c // 2, c % 2
        xc = np.ascontiguousarray(x[b, half * LC:(half + 1) * LC, :])
        if half:
            pf = x[b, :LC, :].sum(axis=0, dtype=np.float64).astype(np.float32)
        else:
            pf = np.zeros(H, dtype=np.float32)
        pfT = np.ascontiguousarray(pf.reshape(KH, P).T)
        in_maps.append({"x": xc, "pf": pfT, "ws": ws, "wo": wo, "ident": ident})

    res = run_bass_kernel_spmd(nc, in_maps, list(range(8)), trace=_trace,
                               **(_trace_kwargs or {}))
    kernel._last_result = res
    kernel._last_in_maps = in_maps
    out = np.empty((B, L, H), dtype=np.float32)
    for c in range(8):
        b, half = c // 2, c % 2
        out[b, half * LC:(half + 1) * LC, :] = res.results[c]["out"]
    return out
